# revision 13
# baseline (speedup 1.0000x reference)
"""Trainium2 Bass kernel for nn_PlainSelfLayer (self-attention with
transpose-before-softmax, returning (selfOutput, attn)).

Reference computation (per batch element b, S=2048, D=1024, A=Dv=512):
    Qp = Q @ Wq.T + bq            [S, A]
    Kp = K @ Wk.T + bk            [S, A]
    WV = V @ Wv.T + bv            [S, Dv]
    S0 = (Qp @ Kp.T) * 1/sqrt(A)  [S(q), S(k)]
    S1 = S0 + mask * (-1e9)
    attn = softmax(S1.T, axis=-1)          # [k, q], normalized over q
    out  = attn @ WV                       # [k, Dv]

Strategy: data-parallel over batch (8 cores, one batch element each).
On-chip layout keeps scores in natural [q-part, k-free] layout:
    P[q, k] = exp(scale*S0 - 1e9*mask)     (no max-subtraction; scores are O(1))
    denom[k] = sum_q P[q, k]               (ones-matmul, PE)
    U[k, v] = sum_q P[q, k] * WV[q, v]     (P natural as lhsT)
    out = U / denom ;  attn = P.T / denom  (PE transpose + scaled copy)
All matmuls run in float32r (full-rate fp32 mode, ~1e-4 relative error).
"""

import math
from contextlib import ExitStack

import numpy as np

import concourse.bass as bass
import concourse.mybir as mybir
import concourse.tile as tile
from concourse import bacc, bass_utils
from concourse.masks import make_identity

F32 = mybir.dt.float32
F32R = mybir.dt.float32r
I32 = mybir.dt.int32

B, S, D, A, DV = 8, 2048, 1024, 512, 512
SCALE = 1.0 / math.sqrt(512.0)
MASKC = -1e9 / SCALE          # mask * MASKC, later multiplied by SCALE in exp
N_CORES = 8

P128 = 128
SD = S // 128                 # 16 s-tiles of 128
DD = D // 128                 # 8 d-chunks
AD = A // 128                 # 4 a-chunks
SC = S // 512                 # 4 s-chunks of 512
KHALF = 4                     # split k range into slabs (SBUF for fp32 P)
KW = S // KHALF               # 1024 k's per half
KCH = KW // 512               # k 512-chunks per half
KTH = KW // 128               # k-tiles (128) per half

TRACE = False                 # set by test harness for profiling


def build_kernel():
    nc = bacc.Bacc("TRN2", target_bir_lowering=False, debug=False)

    dq = nc.dram_tensor("q_in", [S, D], F32R, kind="ExternalInput")
    dk = nc.dram_tensor("k_in", [S, D], F32R, kind="ExternalInput")
    dv = nc.dram_tensor("v_in", [S, D], F32R, kind="ExternalInput")
    dmask = nc.dram_tensor("mask_in", [S, S], I32, kind="ExternalInput")
    dwq = nc.dram_tensor("wq_in", [A, D], F32R, kind="ExternalInput")
    dwk = nc.dram_tensor("wk_in", [A, D], F32R, kind="ExternalInput")
    dwv = nc.dram_tensor("wv_in", [DV, D], F32R, kind="ExternalInput")
    dbq = nc.dram_tensor("bq_in", [A], F32, kind="ExternalInput")
    dbk = nc.dram_tensor("bk_in", [A], F32, kind="ExternalInput")
    dbv = nc.dram_tensor("bv_in", [DV], F32, kind="ExternalInput")
    dattn = nc.dram_tensor("attn_out", [S, S], F32, kind="ExternalOutput")
    dout = nc.dram_tensor("self_out", [S, DV], F32, kind="ExternalOutput")

    with tile.TileContext(nc) as tc, ExitStack() as octx:
        # ---- persistent tiles (whole kernel) ----
        persist = octx.enter_context(tc.tile_pool(name="persist", bufs=1))
        zf = persist.tile([128, 128], F32, name="zf")
        nc.vector.memset(zf[:], 0.0)
        ident_r = persist.tile([128, 128], F32R)
        nc.gpsimd.affine_select(
            out=ident_r[:], in_=zf[:], compare_op=mybir.AluOpType.not_equal,
            fill=1.0, base=0, pattern=[[-1, 128]], channel_multiplier=1)

        # projection outputs (resident)
        qpt = [persist.tile([128, S], F32R, name=f"qpt{a}") for a in range(AD)]
        kpt = [persist.tile([128, S], F32R, name=f"kpt{a}") for a in range(AD)]
        wv = [persist.tile([128, DV], F32R, name=f"wv{st}") for st in range(SD)]

        # biases as per-partition columns (DMAs deferred past first weight loads)
        bq_sb = persist.tile([128, AD], F32)
        bk_sb = persist.tile([128, AD], F32)
        bv_sb = persist.tile([128, DV], F32)
        _bias_emitted = [False]

        def emit_bias_loads():
            if _bias_emitted[0]:
                return
            _bias_emitted[0] = True
            for ac in range(AD):
                nc.gpsimd.dma_start(
                    bq_sb[:, ac:ac + 1],
                    dbq.ap()[ac * 128:(ac + 1) * 128].rearrange("(p o) -> p o", o=1))
                nc.gpsimd.dma_start(
                    bk_sb[:, ac:ac + 1],
                    dbk.ap()[ac * 128:(ac + 1) * 128].rearrange("(p o) -> p o", o=1))
            _bv_ap = dbv.ap()
            nc.gpsimd.dma_start(out=bv_sb[:], in_=bass.AP(
                tensor=_bv_ap.tensor, offset=_bv_ap.offset, ap=[[0, 128], [1, DV]]))

        # ================= Phase 1: projections =================
        with ExitStack() as ctx1:
            wtp = ctx1.enter_context(tc.tile_pool(name="wtp", bufs=6))
            xnat = ctx1.enter_context(tc.tile_pool(name="xnat", bufs=6))
            xtp = ctx1.enter_context(tc.tile_pool(name="xtp", bufs=6))
            tps = ctx1.enter_context(tc.tile_pool(name="tps", bufs=2, space="PSUM"))
            pps = ctx1.enter_context(tc.tile_pool(name="pps", bufs=4, space="PSUM"))

            copy_flip = [0]

            def psum_copy(dst, src):
                """Alternate PSUM->SBUF copies between ACT and DVE."""
                copy_flip[0] ^= 1
                if copy_flip[0]:
                    nc.scalar.copy(dst, src)
                else:
                    nc.vector.tensor_copy(dst, src)

            def transpose_pair(src_tiles, dpair, s_sub_count, tag, pool=None, ptag="xt"):
                """Transpose d-chunks (2*dpair, 2*dpair+1) into one [128, 1024] tile.

                Returns a wide F32R tile: cols [0,512) = d-chunk 2*dpair,
                cols [512,1024) = d-chunk 2*dpair+1 (s runs within each half)."""
                ps = tps.tile([128, 1024], F32R, tag="tps", name=f"tps_{tag}_{dpair}")
                for half in range(2):
                    d = 2 * dpair + half
                    for j in range(s_sub_count):
                        nc.tensor.transpose(
                            ps[:, half * 512 + j * 128:half * 512 + (j + 1) * 128],
                            src_tiles[j][:, d * 128:(d + 1) * 128], ident_r[:])
                xt = (pool or xtp).tile([128, 1024], F32R, tag=ptag,
                                        name=f"xt_{tag}_{dpair}")
                psum_copy(xt[:], ps[:])
                return xt

            def load_and_transpose_weight(dw, na, tag):
                """Load weight [na*128, D] natural; return wide transposed tiles."""
                wn = [xnat.tile([128, D], F32R, tag="wn", name=f"wn_{tag}_{j}")
                      for j in range(na)]
                for j in range(na):
                    nc.sync.dma_start(wn[j][:], dw.ap()[j * 128:(j + 1) * 128, :])
                wts = [transpose_pair(wn, dp, na, f"w{tag}", pool=wtp, ptag="wt")
                       for dp in range(DD // 2)]
                return wts

            def wslice(wts, d, lo, hi):
                return wts[d // 2][:, (d % 2) * 512 + lo:(d % 2) * 512 + hi]

            # --- Q and K projections: produce qpt/kpt [a-part, s-free] ---
            for (dx, dwx, bx, outt, tag) in (
                    (dq, dwq, bq_sb, qpt, "q"), (dk, dwk, bk_sb, kpt, "k")):
                wt = load_and_transpose_weight(dwx, AD, tag)
                emit_bias_loads()
                for sc in range(SC):
                    xn = [xnat.tile([128, D], F32R, tag="xn", name=f"xn_{tag}_{sc}_{j}")
                          for j in range(4)]
                    for j in range(4):
                        nc.sync.dma_start(
                            xn[j][:], dx.ap()[sc * 512 + j * 128:sc * 512 + (j + 1) * 128, :])
                    xts = [transpose_pair(xn, dp, 4, f"{tag}{sc}")
                           for dp in range(DD // 2)]
                    for ac in range(AD):
                        ps = pps.tile([128, 512], F32, tag="pj", name=f"pj_{tag}_{sc}_{ac}")
                        for d in range(DD):
                            nc.tensor.matmul(
                                ps[:], wslice(wt, d, ac * 128, (ac + 1) * 128),
                                wslice(xts, d, 0, 512),
                                start=(d == 0), stop=(d == DD - 1))
                        nc.scalar.add(
                            outt[ac][:, sc * 512:(sc + 1) * 512],
                            ps[:], bx[:, ac:ac + 1])

            # --- V projection: produce wv [s-part, v-free] natural ---
            wtv = load_and_transpose_weight(dwv, AD, "v")
            for sc in range(SC):
                xn = [xnat.tile([128, D], F32R, tag="xn", name=f"xn_v_{sc}_{j}")
                      for j in range(4)]
                for j in range(4):
                    nc.sync.dma_start(
                        xn[j][:], dv.ap()[sc * 512 + j * 128:sc * 512 + (j + 1) * 128, :])
                xts = [transpose_pair(xn, dp, 4, f"v{sc}") for dp in range(DD // 2)]
                for sub in range(4):
                    st = sc * 4 + sub
                    ps = pps.tile([128, 512], F32, tag="pj", name=f"pj_v_{st}")
                    for d in range(DD):
                        nc.tensor.matmul(
                            ps[:], wslice(xts, d, sub * 128, (sub + 1) * 128),
                            wslice(wtv, d, 0, 512),
                            start=(d == 0), stop=(d == DD - 1))
                    nc.vector.tensor_add(wv[st][:], ps[:], bv_sb[:])

        # ================= Phases 2-4 per k-slab =================
        with ExitStack() as ctx2:
            ppool = ctx2.enter_context(tc.tile_pool(name="ppool", bufs=1))
            ptiles = [ppool.tile([128, KW], F32R, name=f"p{i}", bufs=2 * SD,
                                 tag="ptile") for i in range(KHALF * SD)]
            mpool = ctx2.enter_context(tc.tile_pool(name="mpool", bufs=6))
            s2pool = ctx2.enter_context(tc.tile_pool(name="s2pool", bufs=6))
            spsum = ctx2.enter_context(tc.tile_pool(name="spsum", bufs=4, space="PSUM"))
            upsum = ctx2.enter_context(tc.tile_pool(name="upsum", bufs=2, space="PSUM"))
            tpsum = ctx2.enter_context(tc.tile_pool(name="tpsum", bufs=2, space="PSUM"))
            rpool = ctx2.enter_context(tc.tile_pool(name="rpool", bufs=4))
            dnpool = ctx2.enter_context(tc.tile_pool(name="dnpool", bufs=4))
            apool = ctx2.enter_context(tc.tile_pool(name="apool", bufs=2))
            opool = ctx2.enter_context(tc.tile_pool(name="opool", bufs=2))

            for h in range(KHALF):
                k0 = h * KW
                pt = ptiles[h * SD:(h + 1) * SD]  # slot sets ping-pong across slabs

                # --- scores + mask + exp -> P[q-tile][128, KW] ---
                for qt in range(SD):
                    for kc in range(KCH):
                        ps = spsum.tile([128, 512], F32, tag="s",
                                        name=f"s_{h}_{qt}_{kc}")
                        for ac in range(AD):
                            nc.tensor.matmul(
                                ps[:], qpt[ac][:, qt * 128:(qt + 1) * 128],
                                kpt[ac][:, k0 + kc * 512:k0 + (kc + 1) * 512],
                                start=(ac == 0), stop=(ac == AD - 1))
                        mt = mpool.tile([128, 512], I32, tag="m",
                                        name=f"m_{h}_{qt}_{kc}")
                        nc.sync.dma_start(
                            mt[:], dmask.ap()[qt * 128:(qt + 1) * 128,
                                              k0 + kc * 512:k0 + (kc + 1) * 512])
                        s2 = s2pool.tile([128, 512], F32, tag="s2",
                                         name=f"s2_{h}_{qt}_{kc}")
                        nc.vector.scalar_tensor_tensor(
                            out=s2[:], in0=mt[:], scalar=MASKC, in1=ps[:],
                            op0=mybir.AluOpType.mult, op1=mybir.AluOpType.add)
                        nc.scalar.activation(
                            pt[qt][:, kc * 512:(kc + 1) * 512], s2[:],
                            mybir.ActivationFunctionType.Exp, scale=SCALE)

                # --- per k-tile: P.T copies w/ fused denom; U matmul; scale ---
                for kt in range(KTH):
                    # transpose P -> [k, q]; unscaled copy accumulates denom
                    asb = apool.tile([128, S], F32, tag="a", name=f"a_{h}_{kt}")
                    pden = dnpool.tile([128, SC], F32, tag="dn", name=f"dn_{h}_{kt}")
                    for qc in range(SC):
                        tps2 = tpsum.tile([128, 512], F32R, tag="t",
                                          name=f"t_{h}_{kt}_{qc}")
                        for sub in range(4):
                            qt = qc * 4 + sub
                            nc.tensor.transpose(
                                tps2[:, sub * 128:(sub + 1) * 128],
                                pt[qt][:, kt * 128:(kt + 1) * 128],
                                ident_r[:])
                        nc.scalar.activation(
                            asb[:, qc * 512:(qc + 1) * 512], tps2[:],
                            mybir.ActivationFunctionType.Copy,
                            accum_out=pden[:, qc:qc + 1])

                    ups = upsum.tile([128, 512], F32, tag="u", name=f"u_{h}_{kt}")
                    for qt in range(SD):
                        nc.tensor.matmul(ups[:], pt[qt][:, kt * 128:(kt + 1) * 128],
                                         wv[qt][:],
                                         start=(qt == 0), stop=(qt == SD - 1))
                    den = dnpool.tile([128, 1], F32, tag="dsum", name=f"ds_{h}_{kt}")
                    nc.vector.tensor_reduce(
                        out=den[:], in_=pden[:], op=mybir.AluOpType.add,
                        axis=mybir.AxisListType.X)
                    recip = rpool.tile([128, 1], F32, tag="r", name=f"r_{h}_{kt}")
                    nc.vector.reciprocal(recip[:], den[:])

                    # scale attn rows in place (DVE), scale U (ACT), both out
                    nc.gpsimd.tensor_scalar_mul(asb[:], asb[:], recip[:])
                    nc.gpsimd.dma_start(
                        dattn.ap()[k0 + kt * 128:k0 + (kt + 1) * 128, :], asb[:])
                    osb = opool.tile([128, DV], F32, tag="o", name=f"o_{h}_{kt}")
                    nc.scalar.mul(osb[:], ups[:], recip[:])
                    nc.sync.dma_start(
                        dout.ap()[k0 + kt * 128:k0 + (kt + 1) * 128, :], osb[:])

    nc.compile()
    return nc


_NC_CACHE = None


def _get_nc():
    global _NC_CACHE
    if _NC_CACHE is None:
        _NC_CACHE = build_kernel()
    return _NC_CACHE


def run(inputs, trace=False):
    nc = _get_nc()
    in_maps = []
    for b in range(N_CORES):
        in_maps.append({
            "q_in": np.ascontiguousarray(inputs["Q"][b]),
            "k_in": np.ascontiguousarray(inputs["K"][b]),
            "v_in": np.ascontiguousarray(inputs["V"][b]),
            "mask_in": np.ascontiguousarray(inputs["attn_mask"][b]),
            "wq_in": np.ascontiguousarray(inputs["Wq"]),
            "wk_in": np.ascontiguousarray(inputs["Wk"]),
            "wv_in": np.ascontiguousarray(inputs["Wv"]),
            "bq_in": np.ascontiguousarray(inputs["bq"]),
            "bk_in": np.ascontiguousarray(inputs["bk"]),
            "bv_in": np.ascontiguousarray(inputs["bv"]),
        })
    res = bass_utils.run_bass_kernel_spmd(
        nc, in_maps, core_ids=list(range(N_CORES)), trace=trace)
    self_out = np.stack([r["self_out"] for r in res.results])
    attn = np.stack([r["attn_out"] for r in res.results])
    return (self_out, attn), res


def kernel(**inputs):
    (self_out, attn), _ = run(inputs, trace=TRACE)
    return self_out, attn


# revision 14
# speedup vs baseline: 2.0976x; 2.0976x over previous
"""Trainium2 Bass kernel for nn_PlainSelfLayer (self-attention with
transpose-before-softmax, returning (selfOutput, attn)).

Reference computation (per batch element b, S=2048, D=1024, A=Dv=512):
    Qp = Q @ Wq.T + bq            [S, A]
    Kp = K @ Wk.T + bk            [S, A]
    WV = V @ Wv.T + bv            [S, Dv]
    S0 = (Qp @ Kp.T) * 1/sqrt(A)  [S(q), S(k)]
    S1 = S0 + mask * (-1e9)
    attn = softmax(S1.T, axis=-1)          # [k, q], normalized over q
    out  = attn @ WV                       # [k, Dv]

Strategy: data-parallel over batch (8 cores, one batch element each).
On-chip layout keeps scores in natural [q-part, k-free] layout:
    P[q, k] = exp(scale*S0 - 1e9*mask)     (no max-subtraction; scores are O(1))
    denom[k] = sum_q P[q, k]               (ones-matmul, PE)
    U[k, v] = sum_q P[q, k] * WV[q, v]     (P natural as lhsT)
    out = U / denom ;  attn = P.T / denom  (PE transpose + scaled copy)
All matmuls run in float32r (full-rate fp32 mode, ~1e-4 relative error).
"""

import math
from contextlib import ExitStack

import numpy as np

import concourse.bass as bass
import concourse.mybir as mybir
import concourse.tile as tile
from concourse import bacc, bass_utils
from concourse.masks import make_identity

F32 = mybir.dt.float32
F32R = mybir.dt.float32r
I32 = mybir.dt.int32

B, S, D, A, DV = 8, 2048, 1024, 512, 512
SCALE = 1.0 / math.sqrt(512.0)
MASKC = -1e9 / SCALE          # mask * MASKC, later multiplied by SCALE in exp
N_CORES = 8

P128 = 128
SD = S // 128                 # 16 s-tiles of 128
DD = D // 128                 # 8 d-chunks
AD = A // 128                 # 4 a-chunks
SC = S // 512                 # 4 s-chunks of 512
KHALF = 4                     # split k range into slabs (SBUF for fp32 P)
KW = S // KHALF               # 1024 k's per half
KCH = KW // 512               # k 512-chunks per half
KTH = KW // 128               # k-tiles (128) per half

TRACE = False                 # set by test harness for profiling


def build_kernel():
    nc = bacc.Bacc("TRN2", target_bir_lowering=False, debug=False)

    dq = nc.dram_tensor("q_in", [S, D], F32R, kind="ExternalInput")
    dk = nc.dram_tensor("k_in", [S, D], F32R, kind="ExternalInput")
    dv = nc.dram_tensor("v_in", [S, D], F32R, kind="ExternalInput")
    dmask = nc.dram_tensor("mask_in", [S, S], I32, kind="ExternalInput")
    dwq = nc.dram_tensor("wq_in", [A, D], F32R, kind="ExternalInput")
    dwk = nc.dram_tensor("wk_in", [A, D], F32R, kind="ExternalInput")
    dwv = nc.dram_tensor("wv_in", [DV, D], F32R, kind="ExternalInput")
    dbq = nc.dram_tensor("bq_in", [A], F32, kind="ExternalInput")
    dbk = nc.dram_tensor("bk_in", [A], F32, kind="ExternalInput")
    dbv = nc.dram_tensor("bv_in", [DV], F32, kind="ExternalInput")
    dattn = nc.dram_tensor("attn_out", [S, S], F32, kind="ExternalOutput")
    dout = nc.dram_tensor("self_out", [S, DV], F32, kind="ExternalOutput")

    with tile.TileContext(nc) as tc, ExitStack() as octx:
        # ---- persistent tiles (whole kernel) ----
        persist = octx.enter_context(tc.tile_pool(name="persist", bufs=1))
        zf = persist.tile([128, 128], F32, name="zf")
        nc.vector.memset(zf[:], 0.0)
        ident_r = persist.tile([128, 128], F32R)
        nc.gpsimd.affine_select(
            out=ident_r[:], in_=zf[:], compare_op=mybir.AluOpType.not_equal,
            fill=1.0, base=0, pattern=[[-1, 128]], channel_multiplier=1)

        # projection outputs (resident)
        qpt = [persist.tile([128, S], F32R, name=f"qpt{a}") for a in range(AD)]
        kpt = [persist.tile([128, S], F32R, name=f"kpt{a}") for a in range(AD)]
        wv = [persist.tile([128, DV], F32R, name=f"wv{st}") for st in range(SD)]

        # biases as per-partition columns (DMAs deferred past first weight loads)
        bq_sb = persist.tile([128, AD], F32)
        bk_sb = persist.tile([128, AD], F32)
        bv_sb = persist.tile([128, DV], F32)
        _bias_emitted = [False]

        def emit_bias_loads():
            if _bias_emitted[0]:
                return
            _bias_emitted[0] = True
            for ac in range(AD):
                nc.gpsimd.dma_start(
                    bq_sb[:, ac:ac + 1],
                    dbq.ap()[ac * 128:(ac + 1) * 128].rearrange("(p o) -> p o", o=1))
                nc.gpsimd.dma_start(
                    bk_sb[:, ac:ac + 1],
                    dbk.ap()[ac * 128:(ac + 1) * 128].rearrange("(p o) -> p o", o=1))
            _bv_ap = dbv.ap()
            nc.gpsimd.dma_start(out=bv_sb[:], in_=bass.AP(
                tensor=_bv_ap.tensor, offset=_bv_ap.offset, ap=[[0, 128], [1, DV]]))

        # ================= Phase 1: projections =================
        with ExitStack() as ctx1:
            wtp = ctx1.enter_context(tc.tile_pool(name="wtp", bufs=6))
            xnat = ctx1.enter_context(tc.tile_pool(name="xnat", bufs=6))
            xtp = ctx1.enter_context(tc.tile_pool(name="xtp", bufs=6))
            tps = ctx1.enter_context(tc.tile_pool(name="tps", bufs=2, space="PSUM"))
            pps = ctx1.enter_context(tc.tile_pool(name="pps", bufs=4, space="PSUM"))

            copy_flip = [0]

            def psum_copy(dst, src):
                """Alternate PSUM->SBUF copies between ACT and DVE."""
                copy_flip[0] ^= 1
                if copy_flip[0]:
                    nc.scalar.copy(dst, src)
                else:
                    nc.vector.tensor_copy(dst, src)

            def transpose_pair(src_tiles, dpair, s_sub_count, tag, pool=None, ptag="xt"):
                """Transpose d-chunks (2*dpair, 2*dpair+1) into one [128, 1024] tile.

                Returns a wide F32R tile: cols [0,512) = d-chunk 2*dpair,
                cols [512,1024) = d-chunk 2*dpair+1 (s runs within each half)."""
                ps = tps.tile([128, 1024], F32R, tag="tps", name=f"tps_{tag}_{dpair}")
                for half in range(2):
                    d = 2 * dpair + half
                    for j in range(s_sub_count):
                        nc.tensor.transpose(
                            ps[:, half * 512 + j * 128:half * 512 + (j + 1) * 128],
                            src_tiles[j][:, d * 128:(d + 1) * 128], ident_r[:])
                xt = (pool or xtp).tile([128, 1024], F32R, tag=ptag,
                                        name=f"xt_{tag}_{dpair}")
                psum_copy(xt[:], ps[:])
                return xt

            def load_and_transpose_weight(dw, na, tag):
                """Load weight [na*128, D] natural; return wide transposed tiles."""
                wn = [xnat.tile([128, D], F32R, tag="wn", name=f"wn_{tag}_{j}")
                      for j in range(na)]
                for j in range(na):
                    nc.sync.dma_start(wn[j][:], dw.ap()[j * 128:(j + 1) * 128, :])
                wts = [transpose_pair(wn, dp, na, f"w{tag}", pool=wtp, ptag="wt")
                       for dp in range(DD // 2)]
                return wts

            def wslice(wts, d, lo, hi):
                return wts[d // 2][:, (d % 2) * 512 + lo:(d % 2) * 512 + hi]

            # --- Q and K projections: produce qpt/kpt [a-part, s-free] ---
            for (dx, dwx, bx, outt, tag) in (
                    (dq, dwq, bq_sb, qpt, "q"), (dk, dwk, bk_sb, kpt, "k")):
                wt = load_and_transpose_weight(dwx, AD, tag)
                emit_bias_loads()
                for sc in range(SC):
                    xn = [xnat.tile([128, D], F32R, tag="xn", name=f"xn_{tag}_{sc}_{j}")
                          for j in range(4)]
                    for j in range(4):
                        nc.sync.dma_start(
                            xn[j][:], dx.ap()[sc * 512 + j * 128:sc * 512 + (j + 1) * 128, :])
                    xts = [transpose_pair(xn, dp, 4, f"{tag}{sc}")
                           for dp in range(DD // 2)]
                    for ac in range(AD):
                        ps = pps.tile([128, 512], F32, tag="pj", name=f"pj_{tag}_{sc}_{ac}")
                        for d in range(DD):
                            nc.tensor.matmul(
                                ps[:], wslice(wt, d, ac * 128, (ac + 1) * 128),
                                wslice(xts, d, 0, 512),
                                start=(d == 0), stop=(d == DD - 1))
                        nc.scalar.add(
                            outt[ac][:, sc * 512:(sc + 1) * 512],
                            ps[:], bx[:, ac:ac + 1])

            # --- V projection: produce wv [s-part, v-free] natural ---
            wtv = load_and_transpose_weight(dwv, AD, "v")
            for sc in range(SC):
                xn = [xnat.tile([128, D], F32R, tag="xn", name=f"xn_v_{sc}_{j}")
                      for j in range(4)]
                for j in range(4):
                    nc.sync.dma_start(
                        xn[j][:], dv.ap()[sc * 512 + j * 128:sc * 512 + (j + 1) * 128, :])
                xts = [transpose_pair(xn, dp, 4, f"v{sc}") for dp in range(DD // 2)]
                for sub in range(4):
                    st = sc * 4 + sub
                    ps = pps.tile([128, 512], F32, tag="pj", name=f"pj_v_{st}")
                    for d in range(DD):
                        nc.tensor.matmul(
                            ps[:], wslice(xts, d, sub * 128, (sub + 1) * 128),
                            wslice(wtv, d, 0, 512),
                            start=(d == 0), stop=(d == DD - 1))
                    nc.vector.tensor_add(wv[st][:], ps[:], bv_sb[:])

        # ================= Phases 2-4 per k-slab =================
        with ExitStack() as ctx2:
            ppool = ctx2.enter_context(tc.tile_pool(name="ppool", bufs=1))
            ptiles = [ppool.tile([128, KW], F32R, name=f"p{i}", bufs=2 * SD,
                                 tag="ptile") for i in range(KHALF * SD)]
            mpool = ctx2.enter_context(tc.tile_pool(name="mpool", bufs=6))
            s2pool = ctx2.enter_context(tc.tile_pool(name="s2pool", bufs=6))
            spsum = ctx2.enter_context(tc.tile_pool(name="spsum", bufs=4, space="PSUM"))
            upsum = ctx2.enter_context(tc.tile_pool(name="upsum", bufs=2, space="PSUM"))
            tpsum = ctx2.enter_context(tc.tile_pool(name="tpsum", bufs=2, space="PSUM"))
            rpool = ctx2.enter_context(tc.tile_pool(name="rpool", bufs=4))
            dnpool = ctx2.enter_context(tc.tile_pool(name="dnpool", bufs=4))
            apool = ctx2.enter_context(tc.tile_pool(name="apool", bufs=2))
            opool = ctx2.enter_context(tc.tile_pool(name="opool", bufs=2))

            for h in range(KHALF):
                k0 = h * KW
                pt = ptiles[h * SD:(h + 1) * SD]  # slot sets ping-pong across slabs

                # --- scores + mask + exp -> P[q-tile][128, KW] ---
                for qt in range(SD):
                    for kc in range(KCH):
                        ps = spsum.tile([128, 512], F32, tag="s",
                                        name=f"s_{h}_{qt}_{kc}")
                        for ac in range(AD):
                            nc.tensor.matmul(
                                ps[:], qpt[ac][:, qt * 128:(qt + 1) * 128],
                                kpt[ac][:, k0 + kc * 512:k0 + (kc + 1) * 512],
                                start=(ac == 0), stop=(ac == AD - 1))
                        mt = mpool.tile([128, 512], I32, tag="m",
                                        name=f"m_{h}_{qt}_{kc}")
                        nc.sync.dma_start(
                            mt[:], dmask.ap()[qt * 128:(qt + 1) * 128,
                                              k0 + kc * 512:k0 + (kc + 1) * 512])
                        s2 = s2pool.tile([128, 512], F32, tag="s2",
                                         name=f"s2_{h}_{qt}_{kc}")
                        nc.vector.scalar_tensor_tensor(
                            out=s2[:], in0=mt[:], scalar=MASKC, in1=ps[:],
                            op0=mybir.AluOpType.mult, op1=mybir.AluOpType.add)
                        nc.scalar.activation(
                            pt[qt][:, kc * 512:(kc + 1) * 512], s2[:],
                            mybir.ActivationFunctionType.Exp, scale=SCALE)

                # --- per k-tile: P.T copies w/ fused denom; U matmul; scale ---
                for kt in range(KTH):
                    # transpose P -> [k, q]; unscaled copy accumulates denom
                    asb = apool.tile([128, S], F32, tag="a", name=f"a_{h}_{kt}")
                    pden = dnpool.tile([128, SC], F32, tag="dn", name=f"dn_{h}_{kt}")
                    for qc in range(SC):
                        tps2 = tpsum.tile([128, 512], F32R, tag="t",
                                          name=f"t_{h}_{kt}_{qc}")
                        for sub in range(4):
                            qt = qc * 4 + sub
                            nc.tensor.transpose(
                                tps2[:, sub * 128:(sub + 1) * 128],
                                pt[qt][:, kt * 128:(kt + 1) * 128],
                                ident_r[:])
                        nc.scalar.activation(
                            asb[:, qc * 512:(qc + 1) * 512], tps2[:],
                            mybir.ActivationFunctionType.Copy,
                            accum_out=pden[:, qc:qc + 1])

                    ups = upsum.tile([128, 512], F32, tag="u", name=f"u_{h}_{kt}")
                    for qt in range(SD):
                        nc.tensor.matmul(ups[:], pt[qt][:, kt * 128:(kt + 1) * 128],
                                         wv[qt][:],
                                         start=(qt == 0), stop=(qt == SD - 1))
                    den = dnpool.tile([128, 1], F32, tag="dsum", name=f"ds_{h}_{kt}")
                    nc.vector.tensor_reduce(
                        out=den[:], in_=pden[:], op=mybir.AluOpType.add,
                        axis=mybir.AxisListType.X)
                    recip = rpool.tile([128, 1], F32, tag="r", name=f"r_{h}_{kt}")
                    nc.vector.reciprocal(recip[:], den[:])

                    # scale attn rows in place (DVE), scale U (ACT), both out
                    nc.vector.tensor_scalar_mul(asb[:], asb[:], recip[:])
                    nc.gpsimd.dma_start(
                        dattn.ap()[k0 + kt * 128:k0 + (kt + 1) * 128, :], asb[:])
                    osb = opool.tile([128, DV], F32, tag="o", name=f"o_{h}_{kt}")
                    nc.scalar.mul(osb[:], ups[:], recip[:])
                    nc.sync.dma_start(
                        dout.ap()[k0 + kt * 128:k0 + (kt + 1) * 128, :], osb[:])

    nc.compile()
    return nc


_NC_CACHE = None


def _get_nc():
    global _NC_CACHE
    if _NC_CACHE is None:
        _NC_CACHE = build_kernel()
    return _NC_CACHE


def run(inputs, trace=False):
    nc = _get_nc()
    in_maps = []
    for b in range(N_CORES):
        in_maps.append({
            "q_in": np.ascontiguousarray(inputs["Q"][b]),
            "k_in": np.ascontiguousarray(inputs["K"][b]),
            "v_in": np.ascontiguousarray(inputs["V"][b]),
            "mask_in": np.ascontiguousarray(inputs["attn_mask"][b]),
            "wq_in": np.ascontiguousarray(inputs["Wq"]),
            "wk_in": np.ascontiguousarray(inputs["Wk"]),
            "wv_in": np.ascontiguousarray(inputs["Wv"]),
            "bq_in": np.ascontiguousarray(inputs["bq"]),
            "bk_in": np.ascontiguousarray(inputs["bk"]),
            "bv_in": np.ascontiguousarray(inputs["bv"]),
        })
    res = bass_utils.run_bass_kernel_spmd(
        nc, in_maps, core_ids=list(range(N_CORES)), trace=trace)
    self_out = np.stack([r["self_out"] for r in res.results])
    attn = np.stack([r["attn_out"] for r in res.results])
    return (self_out, attn), res


def kernel(**inputs):
    (self_out, attn), _ = run(inputs, trace=TRACE)
    return self_out, attn


# revision 16
# speedup vs baseline: 2.1156x; 1.0086x over previous
"""Trainium2 Bass kernel for nn_PlainSelfLayer (self-attention with
transpose-before-softmax, returning (selfOutput, attn)).

Reference computation (per batch element b, S=2048, D=1024, A=Dv=512):
    Qp = Q @ Wq.T + bq            [S, A]
    Kp = K @ Wk.T + bk            [S, A]
    WV = V @ Wv.T + bv            [S, Dv]
    S0 = (Qp @ Kp.T) * 1/sqrt(A)  [S(q), S(k)]
    S1 = S0 + mask * (-1e9)
    attn = softmax(S1.T, axis=-1)          # [k, q], normalized over q
    out  = attn @ WV                       # [k, Dv]

Strategy: data-parallel over batch (8 cores, one batch element each).
On-chip layout keeps scores in natural [q-part, k-free] layout:
    P[q, k] = exp(scale*S0 - 1e9*mask)     (no max-subtraction; scores are O(1))
    denom[k] = sum_q P[q, k]               (ones-matmul, PE)
    U[k, v] = sum_q P[q, k] * WV[q, v]     (P natural as lhsT)
    out = U / denom ;  attn = P.T / denom  (PE transpose + scaled copy)
All matmuls run in float32r (full-rate fp32 mode, ~1e-4 relative error).
"""

import math
from contextlib import ExitStack

import numpy as np

import concourse.bass as bass
import concourse.mybir as mybir
import concourse.tile as tile
from concourse import bacc, bass_utils
from concourse.masks import make_identity

F32 = mybir.dt.float32
F32R = mybir.dt.float32r
I32 = mybir.dt.int32

B, S, D, A, DV = 8, 2048, 1024, 512, 512
SCALE = 1.0 / math.sqrt(512.0)
MASKC = -1e9 / SCALE          # mask * MASKC, later multiplied by SCALE in exp
N_CORES = 8

P128 = 128
SD = S // 128                 # 16 s-tiles of 128
DD = D // 128                 # 8 d-chunks
AD = A // 128                 # 4 a-chunks
SC = S // 512                 # 4 s-chunks of 512
KHALF = 4                     # split k range into slabs (SBUF for fp32 P)
KW = S // KHALF               # 1024 k's per half
KCH = KW // 512               # k 512-chunks per half
KTH = KW // 128               # k-tiles (128) per half

TRACE = False                 # set by test harness for profiling


def build_kernel():
    nc = bacc.Bacc("TRN2", target_bir_lowering=False, debug=False)

    dq = nc.dram_tensor("q_in", [S, D], F32R, kind="ExternalInput")
    dk = nc.dram_tensor("k_in", [S, D], F32R, kind="ExternalInput")
    dv = nc.dram_tensor("v_in", [S, D], F32R, kind="ExternalInput")
    dmask = nc.dram_tensor("mask_in", [S, S], I32, kind="ExternalInput")
    dwq = nc.dram_tensor("wq_in", [A, D], F32R, kind="ExternalInput")
    dwk = nc.dram_tensor("wk_in", [A, D], F32R, kind="ExternalInput")
    dwv = nc.dram_tensor("wv_in", [DV, D], F32R, kind="ExternalInput")
    dbq = nc.dram_tensor("bq_in", [A], F32, kind="ExternalInput")
    dbk = nc.dram_tensor("bk_in", [A], F32, kind="ExternalInput")
    dbv = nc.dram_tensor("bv_in", [DV], F32, kind="ExternalInput")
    dattn = nc.dram_tensor("attn_out", [S, S], F32, kind="ExternalOutput")
    dout = nc.dram_tensor("self_out", [S, DV], F32, kind="ExternalOutput")

    with tile.TileContext(nc) as tc, ExitStack() as octx:
        # ---- persistent tiles (whole kernel) ----
        persist = octx.enter_context(tc.tile_pool(name="persist", bufs=1))
        zf = persist.tile([128, 128], F32, name="zf")
        nc.vector.memset(zf[:], 0.0)
        ident_r = persist.tile([128, 128], F32R)
        nc.gpsimd.affine_select(
            out=ident_r[:], in_=zf[:], compare_op=mybir.AluOpType.not_equal,
            fill=1.0, base=0, pattern=[[-1, 128]], channel_multiplier=1)

        # projection outputs (resident)
        qpt = [persist.tile([128, S], F32R, name=f"qpt{a}") for a in range(AD)]
        kpt = [persist.tile([128, S], F32R, name=f"kpt{a}") for a in range(AD)]
        wv = [persist.tile([128, DV], F32R, name=f"wv{st}") for st in range(SD)]

        # biases as per-partition columns (DMAs deferred past first weight loads)
        bq_sb = persist.tile([128, AD], F32)
        bk_sb = persist.tile([128, AD], F32)
        bv_sb = persist.tile([128, DV], F32)
        _bias_emitted = [False]

        def emit_bias_loads():
            if _bias_emitted[0]:
                return
            _bias_emitted[0] = True
            for ac in range(AD):
                nc.gpsimd.dma_start(
                    bq_sb[:, ac:ac + 1],
                    dbq.ap()[ac * 128:(ac + 1) * 128].rearrange("(p o) -> p o", o=1))
                nc.gpsimd.dma_start(
                    bk_sb[:, ac:ac + 1],
                    dbk.ap()[ac * 128:(ac + 1) * 128].rearrange("(p o) -> p o", o=1))
            _bv_ap = dbv.ap()
            nc.gpsimd.dma_start(out=bv_sb[:], in_=bass.AP(
                tensor=_bv_ap.tensor, offset=_bv_ap.offset, ap=[[0, 128], [1, DV]]))

        # ================= Phase 1: projections =================
        with ExitStack() as ctx1:
            wtp = ctx1.enter_context(tc.tile_pool(name="wtp", bufs=6))
            xnat = ctx1.enter_context(tc.tile_pool(name="xnat", bufs=6))
            xtp = ctx1.enter_context(tc.tile_pool(name="xtp", bufs=6))
            tps = ctx1.enter_context(tc.tile_pool(name="tps", bufs=2, space="PSUM"))
            pps = ctx1.enter_context(tc.tile_pool(name="pps", bufs=4, space="PSUM"))

            copy_flip = [0]

            def psum_copy(dst, src):
                """Alternate PSUM->SBUF copies between ACT and DVE."""
                copy_flip[0] ^= 1
                if copy_flip[0]:
                    nc.scalar.copy(dst, src)
                else:
                    nc.vector.tensor_copy(dst, src)

            def transpose_pair(src_tiles, dpair, s_sub_count, tag, pool=None, ptag="xt"):
                """Transpose d-chunks (2*dpair, 2*dpair+1) into one [128, 1024] tile.

                Returns a wide F32R tile: cols [0,512) = d-chunk 2*dpair,
                cols [512,1024) = d-chunk 2*dpair+1 (s runs within each half)."""
                ps = tps.tile([128, 1024], F32R, tag="tps", name=f"tps_{tag}_{dpair}")
                for half in range(2):
                    d = 2 * dpair + half
                    for j in range(s_sub_count):
                        nc.tensor.transpose(
                            ps[:, half * 512 + j * 128:half * 512 + (j + 1) * 128],
                            src_tiles[j][:, d * 128:(d + 1) * 128], ident_r[:])
                xt = (pool or xtp).tile([128, 1024], F32R, tag=ptag,
                                        name=f"xt_{tag}_{dpair}")
                psum_copy(xt[:], ps[:])
                return xt

            def load_and_transpose_weight(dw, na, tag):
                """Load weight [na*128, D] natural; return wide transposed tiles."""
                wn = [xnat.tile([128, D], F32R, tag="wn", name=f"wn_{tag}_{j}")
                      for j in range(na)]
                for j in range(na):
                    nc.sync.dma_start(wn[j][:], dw.ap()[j * 128:(j + 1) * 128, :])
                wts = [transpose_pair(wn, dp, na, f"w{tag}", pool=wtp, ptag="wt")
                       for dp in range(DD // 2)]
                return wts

            def wslice(wts, d, lo, hi):
                return wts[d // 2][:, (d % 2) * 512 + lo:(d % 2) * 512 + hi]

            # --- Q and K projections: produce qpt/kpt [a-part, s-free] ---
            for (dx, dwx, bx, outt, tag) in (
                    (dq, dwq, bq_sb, qpt, "q"), (dk, dwk, bk_sb, kpt, "k")):
                wt = load_and_transpose_weight(dwx, AD, tag)
                emit_bias_loads()
                for sc in range(SC):
                    xn = [xnat.tile([128, D], F32R, tag="xn", name=f"xn_{tag}_{sc}_{j}")
                          for j in range(4)]
                    for j in range(4):
                        nc.sync.dma_start(
                            xn[j][:], dx.ap()[sc * 512 + j * 128:sc * 512 + (j + 1) * 128, :])
                    xts = [transpose_pair(xn, dp, 4, f"{tag}{sc}")
                           for dp in range(DD // 2)]
                    for ac in range(AD):
                        ps = pps.tile([128, 512], F32, tag="pj", name=f"pj_{tag}_{sc}_{ac}")
                        for d in range(DD):
                            nc.tensor.matmul(
                                ps[:], wslice(wt, d, ac * 128, (ac + 1) * 128),
                                wslice(xts, d, 0, 512),
                                start=(d == 0), stop=(d == DD - 1))
                        nc.scalar.add(
                            outt[ac][:, sc * 512:(sc + 1) * 512],
                            ps[:], bx[:, ac:ac + 1])

            # --- V projection: produce wv [s-part, v-free] natural ---
            wtv = load_and_transpose_weight(dwv, AD, "v")
            for sc in range(SC):
                xn = [xnat.tile([128, D], F32R, tag="xn", name=f"xn_v_{sc}_{j}")
                      for j in range(4)]
                for j in range(4):
                    nc.sync.dma_start(
                        xn[j][:], dv.ap()[sc * 512 + j * 128:sc * 512 + (j + 1) * 128, :])
                xts = [transpose_pair(xn, dp, 4, f"v{sc}") for dp in range(DD // 2)]
                for sub in range(4):
                    st = sc * 4 + sub
                    ps = pps.tile([128, 512], F32, tag="pj", name=f"pj_v_{st}")
                    for d in range(DD):
                        nc.tensor.matmul(
                            ps[:], wslice(xts, d, sub * 128, (sub + 1) * 128),
                            wslice(wtv, d, 0, 512),
                            start=(d == 0), stop=(d == DD - 1))
                    nc.vector.tensor_add(wv[st][:], ps[:], bv_sb[:])

        # ================= Phases 2-4 per k-slab =================
        with ExitStack() as ctx2:
            ppool = ctx2.enter_context(tc.tile_pool(name="ppool", bufs=1))
            ptiles = [ppool.tile([128, KW], F32R, name=f"p{i}", bufs=2 * SD,
                                 tag="ptile") for i in range(KHALF * SD)]
            mpool = ctx2.enter_context(tc.tile_pool(name="mpool", bufs=6))
            s2pool = ctx2.enter_context(tc.tile_pool(name="s2pool", bufs=6))
            spsum = ctx2.enter_context(tc.tile_pool(name="spsum", bufs=4, space="PSUM"))
            upsum = ctx2.enter_context(tc.tile_pool(name="upsum", bufs=2, space="PSUM"))
            tpsum = ctx2.enter_context(tc.tile_pool(name="tpsum", bufs=2, space="PSUM"))
            rpool = ctx2.enter_context(tc.tile_pool(name="rpool", bufs=4))
            dnpool = ctx2.enter_context(tc.tile_pool(name="dnpool", bufs=4))
            apool = ctx2.enter_context(tc.tile_pool(name="apool", bufs=2))
            opool = ctx2.enter_context(tc.tile_pool(name="opool", bufs=2))

            for h in range(KHALF):
                k0 = h * KW
                pt = ptiles[h * SD:(h + 1) * SD]  # slot sets ping-pong across slabs

                # --- scores + mask + exp -> P[q-tile][128, KW] ---
                for qt in range(SD):
                    for kc in range(KCH):
                        ps = spsum.tile([128, 512], F32, tag="s",
                                        name=f"s_{h}_{qt}_{kc}")
                        for ac in range(AD):
                            nc.tensor.matmul(
                                ps[:], qpt[ac][:, qt * 128:(qt + 1) * 128],
                                kpt[ac][:, k0 + kc * 512:k0 + (kc + 1) * 512],
                                start=(ac == 0), stop=(ac == AD - 1))
                        mt = mpool.tile([128, 512], I32, tag="m",
                                        name=f"m_{h}_{qt}_{kc}")
                        nc.sync.dma_start(
                            mt[:], dmask.ap()[qt * 128:(qt + 1) * 128,
                                              k0 + kc * 512:k0 + (kc + 1) * 512])
                        s2 = s2pool.tile([128, 512], F32, tag="s2",
                                         name=f"s2_{h}_{qt}_{kc}")
                        nc.vector.scalar_tensor_tensor(
                            out=s2[:], in0=mt[:], scalar=MASKC, in1=ps[:],
                            op0=mybir.AluOpType.mult, op1=mybir.AluOpType.add)
                        nc.scalar.activation(
                            pt[qt][:, kc * 512:(kc + 1) * 512], s2[:],
                            mybir.ActivationFunctionType.Exp, scale=SCALE)

                # --- per k-tile: P.T copies w/ fused denom; U matmul; scale ---
                for kt in range(KTH):
                    # transpose P -> [k, q]; unscaled copy accumulates denom
                    asb = apool.tile([128, S], F32, tag="a", name=f"a_{h}_{kt}")
                    pden = dnpool.tile([128, SC], F32, tag="dn", name=f"dn_{h}_{kt}")
                    for qc in range(SC):
                        tps2 = tpsum.tile([128, 512], F32R, tag="t",
                                          name=f"t_{h}_{kt}_{qc}")
                        for sub in range(4):
                            qt = qc * 4 + sub
                            nc.tensor.transpose(
                                tps2[:, sub * 128:(sub + 1) * 128],
                                pt[qt][:, kt * 128:(kt + 1) * 128],
                                ident_r[:])
                        nc.vector.tensor_scalar(
                            out=asb[:, qc * 512:(qc + 1) * 512], in0=tps2[:],
                            scalar1=1.0, scalar2=0.0,
                            op0=mybir.AluOpType.mult, op1=mybir.AluOpType.add,
                            accum_out=pden[:, qc:qc + 1])

                    ups = upsum.tile([128, 512], F32, tag="u", name=f"u_{h}_{kt}")
                    for qt in range(SD):
                        nc.tensor.matmul(ups[:], pt[qt][:, kt * 128:(kt + 1) * 128],
                                         wv[qt][:],
                                         start=(qt == 0), stop=(qt == SD - 1))
                    den = dnpool.tile([128, 1], F32, tag="dsum", name=f"ds_{h}_{kt}")
                    nc.vector.tensor_reduce(
                        out=den[:], in_=pden[:], op=mybir.AluOpType.add,
                        axis=mybir.AxisListType.X)
                    recip = rpool.tile([128, 1], F32, tag="r", name=f"r_{h}_{kt}")
                    nc.vector.reciprocal(recip[:], den[:])

                    # scale attn rows in place (DVE), scale U (ACT), both out
                    nc.vector.tensor_scalar_mul(asb[:], asb[:], recip[:])
                    nc.gpsimd.dma_start(
                        dattn.ap()[k0 + kt * 128:k0 + (kt + 1) * 128, :], asb[:])
                    osb = opool.tile([128, DV], F32, tag="o", name=f"o_{h}_{kt}")
                    nc.scalar.mul(osb[:], ups[:], recip[:])
                    nc.sync.dma_start(
                        dout.ap()[k0 + kt * 128:k0 + (kt + 1) * 128, :], osb[:])

    nc.compile()
    return nc


_NC_CACHE = None


def _get_nc():
    global _NC_CACHE
    if _NC_CACHE is None:
        _NC_CACHE = build_kernel()
    return _NC_CACHE


def run(inputs, trace=False):
    nc = _get_nc()
    in_maps = []
    for b in range(N_CORES):
        in_maps.append({
            "q_in": np.ascontiguousarray(inputs["Q"][b]),
            "k_in": np.ascontiguousarray(inputs["K"][b]),
            "v_in": np.ascontiguousarray(inputs["V"][b]),
            "mask_in": np.ascontiguousarray(inputs["attn_mask"][b]),
            "wq_in": np.ascontiguousarray(inputs["Wq"]),
            "wk_in": np.ascontiguousarray(inputs["Wk"]),
            "wv_in": np.ascontiguousarray(inputs["Wv"]),
            "bq_in": np.ascontiguousarray(inputs["bq"]),
            "bk_in": np.ascontiguousarray(inputs["bk"]),
            "bv_in": np.ascontiguousarray(inputs["bv"]),
        })
    res = bass_utils.run_bass_kernel_spmd(
        nc, in_maps, core_ids=list(range(N_CORES)), trace=trace)
    self_out = np.stack([r["self_out"] for r in res.results])
    attn = np.stack([r["attn_out"] for r in res.results])
    return (self_out, attn), res


def kernel(**inputs):
    (self_out, attn), _ = run(inputs, trace=TRACE)
    return self_out, attn
